# revision 2
# baseline (speedup 1.0000x reference)
"""Distributed EnhancedResGCN forward for 8 Trainium2 NeuronCores — v2.

Major changes over v1 (baseline):
  - All gather tables are bf16 with 256B rows: P = [hs|h] [N,128]; z/d tables
    [N,128] with a zero top half. Gather DMA bytes halve; PE runs bf16
    (1 cyc/row vs fp32's 4).
  - Per-node state kept FEATURE-major in two persistent SBUF tiles
    (HB = [h^T; nb^T], AGG = sonh/h_lin), so attention + BN + update run as
    25 wide [64,512] chunk ops instead of ~98 serial per-block chains.
  - One-hot masks built bf16 in fused multi-tile DVE ops (3D broadcast),
    ~3 ops per phase instead of one per tile.
  - Phases of ~120 tiles -> 4 gather calls per phase (68 per SpMM instead
    of 392), amortizing the ~1us SWDGE fixed overhead.
  - BN statistics computed AFTER the linear projection via row-reduces
    (sum/sumsq per feature = per-partition), removing the covariance-matrix
    AllReduce trick entirely; the AR payload is [64,2].
  - Matmuls are "flipped": lhsT = gathered tile (stationary), rhs = mask,
    accumulating feature-major sums directly into per-phase PSUM column
    slices; epilogues are two wide copies per phase.
"""

import os
import sys

for _p in ("/opt/trn_rl_repo", "/root/.axon_site/_ro/trn_rl_repo"):
    if os.path.isdir(_p) and _p not in sys.path:
        sys.path.append(_p)

import numpy as np

import concourse.bass as bass
import concourse.tile as tile
from concourse import bacc, mybir
from concourse.bass_utils import run_bass_kernel_spmd

try:
    import bass_rust
except ImportError:  # pragma: no cover
    bass_rust = None

import ml_dtypes

F32 = mybir.dt.float32
BF16 = mybir.dt.float16  # fp16: 8x finer mantissa than bf16, same engine rates
I16 = mybir.dt.int16
AF = mybir.ActivationFunctionType
OP = mybir.AluOpType

NCORES = 8
BLK = 128  # node-block granularity for table exports (transposes)
SLOTW = 80  # dst-slot width of one-hot masks / PSUM accumulators
NACC = 5  # concurrent accumulation groups (PSUM banks); phase nb <= 2*NACC
BUCKET = 25000
MAX_PHASE_TILES = int(os.environ.get("GCN_MPT", "110"))
MASKW = 40  # mask chunk capacity (tiles per bucket-half run)
CHUNK = 512
EPS = 1e-5
STAGE = int(os.environ.get("GCN_STAGE", "99"))

bf16 = np.float16  # host-side 2-byte dtype matching mybir.dt.float16


# --------------------------------------------------------------------------
# walrus in this container rejects instructions carrying >1 sem wait; hoist
# extras onto same-engine NOPs inserted right before the instruction.
def _split_excess_waits(nc, max_waits=1):
    n_split = 0
    for fn in nc.m.functions:
        for blk in fn.blocks:
            insts = blk.instructions
            pos = 0
            while pos < len(insts):
                inst = insts[pos]
                si = inst.sync_info
                waits = list(si.on_wait) if si is not None and si.on_wait else []
                if len(waits) > max_waits:
                    si.on_wait = waits[:max_waits]
                    extra = waits[max_waits:]
                    at = pos
                    for j in range(0, len(extra), max_waits):
                        nop = mybir.InstNoOp(
                            name=f"waitnop_{n_split}_{j}", ins=[], outs=[]
                        )
                        nop.engine = inst.engine
                        nop.sync_info = bass_rust.SyncInfo(
                            on_wait=extra[j : j + max_waits], on_update=[]
                        )
                        try:
                            nc.register_instruction(nop, overwrite=True)
                        except Exception:
                            pass
                        insts.insert(at, nop)
                        at += 1
                        pos += 1
                    n_split += 1
                pos += 1
    return n_split


# --------------------------------------------------------------------------
# host-side edge preprocessing (same tiling scheme as v1: edges grouped by
# (dst block, src bucket), padded to 128-edge tiles, structure uniform
# across cores; phases pack whole blocks up to MAX_PHASE_TILES tiles)
def _preprocess(N, src, dst):
    S = N // NCORES
    NBLK = (S + BLK - 1) // BLK
    NSB0 = (NBLK * BLK + SLOTW - 1) // SLOTW
    S_pad = max(NBLK * BLK, NSB0 * SLOTW)
    nbuck = (N + BUCKET - 1) // BUCKET
    bucket_sizes = [min(BUCKET, N - j * BUCKET) for j in range(nbuck)]

    in_deg = np.bincount(dst, minlength=N).astype(np.float64)
    out_deg = np.bincount(src, minlength=N).astype(np.float64)
    in_deg_c = np.maximum(in_deg, 1.0)
    out_deg_c = np.maximum(out_deg, 1.0)
    in_norm = (in_deg_c**-0.5).astype(np.float32)
    out_norm = (out_deg_c**-0.5).astype(np.float32)
    r_indeg = (1.0 / in_deg_c).astype(np.float32)

    NSB = NSB0  # dst sub-blocks of SLOTW nodes
    percore = []
    counts = np.zeros((NCORES, NSB * nbuck), np.int64)
    for c in range(NCORES):
        m = (dst >= c * S) & (dst < (c + 1) * S)
        es = src[m]
        ed = dst[m] - c * S
        bl = ed // SLOTW
        bu = es // BUCKET
        key = bl * nbuck + bu
        order = np.lexsort((es, key))
        es, ed, key = es[order], ed[order], key[order]
        starts = np.searchsorted(key, np.arange(NSB * nbuck))
        ends = np.searchsorted(key, np.arange(NSB * nbuck) + 1)
        counts[c] = ends - starts
        percore.append((es, ed, starts, ends))

    maxc = counts.max(axis=0).reshape(NSB, nbuck)
    tiles_per = (maxc + BLK - 1) // BLK  # 128-edge tiles per (sub-block, bucket)
    empty = tiles_per.sum(axis=1) == 0
    tiles_per[empty, 0] = 1

    blk_tiles = tiles_per.sum(axis=1)
    phases = []
    cur, cur_t = [], 0
    for b in range(NSB):
        if cur and (cur_t + blk_tiles[b] > MAX_PHASE_TILES or len(cur) >= 2 * NACC):
            phases.append(cur)
            cur, cur_t = [], 0
        cur.append(b)
        cur_t += blk_tiles[b]
    if cur:
        phases.append(cur)

    # Global tile order: phase -> bucket -> sub-block.  Matmuls execute in
    # half-major order (sub-blocks [0,NACC) of the phase first, then the
    # rest) so only NACC accumulation groups are live at once — each group
    # needs its own PSUM bank.  Masks are built per (bucket, half) run,
    # which is a contiguous pcol range.
    phase_meta = []
    T = 0
    for blist in phases:
        nb = len(blist)
        nbA = min(nb, NACC)
        calls = []
        pcol = 0
        tinfo = {0: [], 1: []}  # half -> [(pcol, bi, gtile)]
        runs = {0: [], 1: []}  # half -> [(pcol_start, width)]
        first_last = {}
        for j in range(nbuck):
            cnt = int(sum(tiles_per[b, j] for b in blist))
            if cnt == 0:
                continue
            calls.append(dict(bucket=j, off=pcol, cnt=cnt, gtile=T + pcol))
            for half in (0, 1):
                r0 = pcol
                bset = range(nbA) if half == 0 else range(nbA, nb)
                for bi in bset:
                    b = blist[bi]
                    for _ in range(int(tiles_per[b, j])):
                        tinfo[half].append([pcol, bi, T + pcol])
                        if bi not in first_last:
                            first_last[bi] = [pcol, pcol]
                        else:
                            first_last[bi][1] = pcol
                        pcol += 1
                if pcol > r0:
                    runs[half].append((r0, pcol - r0))
        halves = []
        for half in (0, 1):
            tiles = []
            # map each tile to its (run index, offset) for mask lookup
            run_of = {}
            for ri, (r0, wdt) in enumerate(runs[half]):
                for p in range(r0, r0 + wdt):
                    run_of[p] = (ri, p - r0)
            for pc, bi, gt in tinfo[half]:
                ri, off = run_of[pc]
                tiles.append(
                    dict(
                        pcol=pc,
                        bi=bi,
                        gtile=gt,
                        run=ri,
                        off=off,
                        start=(pc == first_last[bi][0]),
                        stop=(pc == first_last[bi][1]),
                    )
                )
            halves.append(dict(tiles=tiles, runs=runs[half], bi0=0 if half == 0 else nbA))
        assert max((w for _, w in runs[0] + runs[1]), default=0) <= MASKW
        phase_meta.append(
            dict(
                blocks=[dict(b=b) for b in blist],
                ntiles=pcol,
                calls=calls,
                halves=halves,
                b0=blist[0],
                nb=nb,
                nbA=nbA,
            )
        )
        T += pcol

    # per-core idx (int16) in global order; seg = dst slot in block (bf16),
    # 255 for pad lanes
    idx_all = np.zeros((NCORES, T, 128), np.int16)
    seg_all = np.full((NCORES, T, 128), 255.0, np.float32)
    for c in range(NCORES):
        es, ed, starts, ends = percore[c]
        t_cursor = 0
        for ph in phase_meta:
            for call in ph["calls"]:
                j = call["bucket"]
                for binfo in ph["blocks"]:
                    b = binfo["b"]
                    nt = int(tiles_per[b, j])
                    if nt == 0:
                        continue
                    g = b * nbuck + j
                    s, e = int(starts[g]), int(ends[g])
                    cnt = e - s
                    loc_idx = (es[s:e] - j * BUCKET).astype(np.int16)
                    loc_seg = (ed[s:e] - b * SLOTW).astype(np.float32)
                    pad_idx = loc_idx[-1] if cnt > 0 else np.int16(0)
                    block_idx = np.full(nt * 128, pad_idx, np.int16)
                    block_seg = np.full(nt * 128, 255.0, np.float32)
                    block_idx[:cnt] = loc_idx
                    block_seg[:cnt] = loc_seg
                    idx_all[c, t_cursor : t_cursor + nt] = block_idx.reshape(nt, 128)
                    seg_all[c, t_cursor : t_cursor + nt] = block_seg.reshape(nt, 128)
                    t_cursor += nt
        assert t_cursor == T

    # idx wrap: [128] -> [16, 8] (pos i -> [i%16, i//16]), replicate to 128
    idx_wrapped = np.zeros((NCORES, 128, T * 8), np.int16)
    w = idx_all.reshape(NCORES, T, 8, 16).transpose(0, 3, 1, 2).reshape(NCORES, 16, T * 8)
    for r in range(8):
        idx_wrapped[:, 16 * r : 16 * (r + 1), :] = w
    seg_cols = seg_all.transpose(0, 2, 1).copy()  # [NCORES, 128, T]

    def rep_rows(vec):
        """[N] per-node scalar -> per-core [128, S_pad] bf16, all rows equal,
        pad cols zero."""
        out = np.zeros((NCORES, 128, S_pad), bf16)
        for c in range(NCORES):
            v = np.zeros(S_pad, np.float32)
            v[:S] = vec[c * S : (c + 1) * S]
            out[c] = np.broadcast_to(v.astype(bf16), (128, S_pad))
        return out

    return dict(
        N=N,
        S=S,
        NBLK=NBLK,
        S_pad=S_pad,
        nbuck=nbuck,
        bucket_sizes=bucket_sizes,
        phases=phase_meta,
        T=T,
        idx=idx_wrapped,
        seg=seg_cols,
        rindeg_rep=rep_rows(r_indeg),
        innorm_rep=rep_rows(in_norm),
        outnorm_rep=rep_rows(out_norm),
    )


# --------------------------------------------------------------------------
def _build_program(meta, IN, H, C, L, reps=1):
    N, S, NBLK, S_pad = meta["N"], meta["S"], meta["NBLK"], meta["S_pad"]
    T = meta["T"]
    n_inv = 1.0 / N
    NCH = (S_pad + CHUNK - 1) // CHUNK

    nc = bacc.Bacc("TRN2", target_bir_lowering=False, debug=False, num_devices=NCORES)

    # ---- I/O ----
    featT_d = nc.dram_tensor("featT", [IN, S], BF16, kind="ExternalInput")
    idx_d = nc.dram_tensor("idx", [128, T * 8], I16, kind="ExternalInput")
    seg_d = nc.dram_tensor("seg", [128, T], BF16, kind="ExternalInput")
    iota_d = nc.dram_tensor("iota", [128, 128], BF16, kind="ExternalInput")
    identb_d = nc.dram_tensor("identb", [128, 128], BF16, kind="ExternalInput")
    ones1_d = nc.dram_tensor("ones1", [1, 128], BF16, kind="ExternalInput")
    rr_d = nc.dram_tensor("rrep", [128, S_pad], BF16, kind="ExternalInput")
    ir_d = nc.dram_tensor("irep", [128, S_pad], BF16, kind="ExternalInput")
    or_d = nc.dram_tensor("orep", [128, S_pad], BF16, kind="ExternalInput")
    wenc1_d = nc.dram_tensor("wenc1", [IN, IN // 2], BF16, kind="ExternalInput")
    benc1_d = nc.dram_tensor("benc1", [IN // 2, 1], F32, kind="ExternalInput")
    wenc2_d = nc.dram_tensor("wenc2", [IN // 2, IN // 4], BF16, kind="ExternalInput")
    benc2_d = nc.dram_tensor("benc2", [IN // 4, 1], F32, kind="ExternalInput")
    wenc3_d = nc.dram_tensor("wenc3", [IN // 4, IN], BF16, kind="ExternalInput")
    benc3_d = nc.dram_tensor("benc3", [IN, 1], F32, kind="ExternalInput")
    w0_d = nc.dram_tensor("w0", [IN, H], BF16, kind="ExternalInput")
    wr_d = nc.dram_tensor("wrest", [H, (L - 1) * H], BF16, kind="ExternalInput")
    gam_d = nc.dram_tensor("gam", [H, L], F32, kind="ExternalInput")
    bet_d = nc.dram_tensor("bet", [H, L], F32, kind="ExternalInput")
    watt1_d = nc.dram_tensor("watt1", [2 * H, H], BF16, kind="ExternalInput")
    batt1_d = nc.dram_tensor("batt1", [H, 1], F32, kind="ExternalInput")
    watt2_d = nc.dram_tensor("watt2", [H, 1], BF16, kind="ExternalInput")
    batt2_d = nc.dram_tensor("batt2", [1, 1], F32, kind="ExternalInput")
    wfc_d = nc.dram_tensor("wfc", [H, C], BF16, kind="ExternalInput")
    bfc_d = nc.dram_tensor("bfc", [C, 1], F32, kind="ExternalInput")
    outT_d = nc.dram_tensor("outT", [C, S], F32, kind="ExternalOutput")

    NDUMP = int(os.environ.get("GCN_DUMP", "0"))
    dbg_d = [
        nc.dram_tensor(f"dbg{i}", [128, S_pad], BF16, kind="ExternalOutput")
        for i in range(NDUMP)
    ]
    dump_state = {"i": 0}
    DUMPG = os.environ.get("GCN_DUMPG")
    if DUMPG:
        dbgz_d = nc.dram_tensor("dbgz", [S, 128], BF16, kind="ExternalOutput")

    AS = os.environ.get("GCN_AS", "Shared")
    P_local = nc.dram_tensor("P_local", [S, 128], BF16)
    P_full = nc.dram_tensor("P_full", [N, 128], BF16, addr_space=AS)
    g_local = nc.dram_tensor("g_local", [S, 128], BF16)
    g_full = nc.dram_tensor("g_full", [N, 128], BF16, addr_space=AS)
    st_local = nc.dram_tensor("st_local", [H, 2], F32)
    st_full = nc.dram_tensor("st_full", [H, 2], F32, addr_space=AS)

    RG = [list(range(NCORES))]

    with tile.TileContext(nc) as tc:
        import contextlib

        ctx = contextlib.ExitStack()
        const = ctx.enter_context(tc.tile_pool(name="const", bufs=1))
        persist = ctx.enter_context(tc.tile_pool(name="persist", bufs=1))
        gath = ctx.enter_context(tc.tile_pool(name="gath", bufs=2))
        mpool = ctx.enter_context(tc.tile_pool(name="mpool", bufs=2))
        reppool = ctx.enter_context(tc.tile_pool(name="reppool", bufs=2))
        stage = ctx.enter_context(tc.tile_pool(name="stage", bufs=2))
        small = ctx.enter_context(tc.tile_pool(name="small", bufs=2))
        # PSUM is bank-granular (8 x 2KB/partition); every live accumulation
        # group needs its own bank (interleaved start/stop groups in one bank
        # lose partials).  5 acc banks + 2 matmul + 1 transpose = 8.
        pacc = ctx.enter_context(tc.tile_pool(name="pacc", bufs=1, space="PSUM"))
        pmm = ctx.enter_context(tc.tile_pool(name="pmm", bufs=2, space="PSUM"))
        ptr = ctx.enter_context(tc.tile_pool(name="ptr", bufs=1, space="PSUM"))

        def load_const(dram, shape, dtype=F32, name=None):
            t = const.tile(shape, dtype, name=name or dram.name + "_s")
            nc.sync.dma_start(out=t[:], in_=dram[:])
            return t

        idx_s = load_const(idx_d, [128, T * 8], I16)
        seg_s = load_const(seg_d, [128, T], BF16)
        iota = load_const(iota_d, [128, 128], BF16)
        identb = load_const(identb_d, [128, 128], BF16)
        ones1 = load_const(ones1_d, [1, 128], BF16)
        wenc1 = load_const(wenc1_d, [IN, IN // 2], BF16)
        benc1 = load_const(benc1_d, [IN // 2, 1])
        wenc2 = load_const(wenc2_d, [IN // 2, IN // 4], BF16)
        benc2 = load_const(benc2_d, [IN // 4, 1])
        wenc3 = load_const(wenc3_d, [IN // 4, IN], BF16)
        benc3 = load_const(benc3_d, [IN, 1])
        w0 = load_const(w0_d, [IN, H], BF16)
        wrest = load_const(wr_d, [H, (L - 1) * H], BF16)
        gam = load_const(gam_d, [H, L])
        bet = load_const(bet_d, [H, L])
        watt1 = load_const(watt1_d, [2 * H, H], BF16)
        batt1 = load_const(batt1_d, [H, 1])
        watt2 = load_const(watt2_d, [H, 1], BF16)
        batt2 = load_const(batt2_d, [1, 1])
        wfc = load_const(wfc_d, [H, C], BF16)
        bfc = load_const(bfc_d, [C, 1])

        # persistent node state, feature-major
        # HB rows 0:H = h^T, rows H:2H = S_h / nb^T
        HB = persist.tile([128, S_pad], BF16, name="HB")
        # AGG rows 0:H = sonh -> +delta -> h_lin
        AGG = persist.tile([128, S_pad], BF16, name="AGG")

        # zero halves of g_local that exports never write (cols H:128)
        zt = const.tile([128, 128], BF16, name="zerot")
        nc.vector.memset(zt[:], 0.0)
        for b in range(NBLK):
            bs = min(BLK, S - b * BLK)
            if bs <= 0:
                break
            nc.sync.dma_start(
                out=g_local[b * BLK : b * BLK + bs, H:128], in_=zt[:bs, :H]
            )

        def bs_of(b):
            return min(BLK, S - b * BLK)

        def dump(tile_ap):
            i = dump_state["i"]
            if i < NDUMP:
                nc.sync.dma_start(out=dbg_d[i][:], in_=tile_ap)
                dump_state["i"] = i + 1

        # ----------------------------------------------------------------
        def spmm(table_dram, full_lhs, accum, rep, tag):
            """One SpMM over all edges.

            full_lhs: True -> lhsT = gathered [128] cols (pass A over P),
                      acc rows 0:2H; False -> lhsT = cols 0:H (z/d tables),
                      acc rows 0:H.
            accum: 'copy' (L0 pass) or 'add' (pass B) or 'passA'.
            """
            for pi, ph in enumerate(meta["phases"]):
                nb = ph["nb"]
                cols0 = ph["b0"] * SLOTW
                g = gath.tile(
                    [128, MAX_PHASE_TILES, 128], BF16, tag="gath",
                    name=f"g_{tag}_{rep}_{pi}",
                )
                for call in ph["calls"]:
                    j = call["bucket"]
                    bsz = meta["bucket_sizes"][j]
                    off, cnt, gt = call["off"], call["cnt"], call["gtile"]
                    nc.gpsimd.dma_gather(
                        g[:, off : off + cnt, :],
                        table_dram[j * BUCKET : j * BUCKET + bsz, :],
                        idx_s[:, gt * 8 : (gt + cnt) * 8],
                        cnt * 128,
                        cnt * 128,
                        128,
                        single_packet=False,
                    )
                # Interleaved start/stop accumulation groups sharing a PSUM
                # bank lose partials (HW zeroes beyond the written slice), so
                # every live group gets its own bank-backed tile; the phase's
                # sub-blocks run in two halves of <= NACC groups each.
                gt0 = ph["calls"][0]["gtile"] - ph["calls"][0]["off"]
                for hf, half in enumerate(ph["halves"]):
                    if not half["tiles"]:
                        continue
                    bi0 = half["bi0"]
                    nbh = (ph["nbA"] if hf == 0 else ph["nb"] - ph["nbA"])
                    accs = [
                        pacc.tile([128, SLOTW], F32, tag=f"acc{k}",
                                  name=f"acc_{tag}_{rep}_{pi}_{hf}_{k}")
                        for k in range(nbh)
                    ]
                    masks = []
                    for ri, (r0, wdt) in enumerate(half["runs"]):
                        m = mpool.tile(
                            [128, MASKW, SLOTW], BF16, tag="mask",
                            name=f"m_{tag}_{rep}_{pi}_{hf}_{ri}",
                        )
                        nc.vector.tensor_tensor(
                            out=m[:, :wdt, :],
                            in0=iota[:, None, :SLOTW].to_broadcast([128, wdt, SLOTW]),
                            in1=seg_s[:, gt0 + r0 : gt0 + r0 + wdt, None].to_broadcast(
                                [128, wdt, SLOTW]
                            ),
                            op=OP.is_equal,
                        )
                        masks.append(m)
                    for ti in half["tiles"]:
                        pc, bi = ti["pcol"], ti["bi"]
                        lhs = g[:, pc, :] if full_lhs else g[:, pc, :H]
                        nc.tensor.matmul(
                            out=accs[bi - bi0][: (128 if full_lhs else H), :],
                            lhsT=lhs,
                            rhs=masks[ti["run"]][:, ti["off"], :],
                            start=ti["start"],
                            stop=ti["stop"],
                        )
                    for k in range(nbh):
                        bi = bi0 + k
                        cols = slice(cols0 + bi * SLOTW, cols0 + (bi + 1) * SLOTW)
                        acc = accs[k]
                        if accum == "passA":
                            nc.scalar.activation(
                                out=AGG[:H, cols], in_=acc[:H, :], func=AF.Identity
                            )
                            nc.scalar.activation(
                                out=HB[H:128, cols], in_=acc[H:128, :], func=AF.Identity
                            )
                        elif accum == "copy":
                            nc.scalar.activation(
                                out=AGG[:H, cols], in_=acc[:H, :], func=AF.Identity
                            )
                        else:  # add
                            nc.vector.tensor_tensor(
                                out=AGG[:H, cols], in0=AGG[:H, cols], in1=acc[:H, :],
                                op=OP.add,
                            )

        # ----------------------------------------------------------------
        def chunk_cols(ci):
            c0 = ci * CHUNK
            return c0, min(CHUNK, S_pad - c0)

        def export_P(hsrc_ap, hs_tile, c0, w, li):
            """hsrc_ap: h^T [H, w] bf16 (HB slice); hs_tile: hs^T [H, w].
            Writes P_local rows [c0, c0+w) with layout [hs | h]."""
            nblk_w = (w + BLK - 1) // BLK
            for k in range(nblk_w):
                b = (c0 + k * BLK) // BLK
                bs = bs_of(b)
                if bs <= 0:
                    break
                tp = ptr.tile([128, 128], BF16, tag="ptr", name=f"pT{li}_{b}")
                nc.tensor.transpose(
                    out=tp[:, :H], in_=hs_tile[:, k * BLK : k * BLK + 128],
                    identity=identb[:H, :H],
                )
                nc.tensor.transpose(
                    out=tp[:, H:128], in_=hsrc_ap[:, k * BLK : k * BLK + 128],
                    identity=identb[:H, :H],
                )
                st = stage.tile([128, 128], BF16, tag="pexp", name=f"pS{li}_{b}")
                nc.vector.tensor_copy(out=st[:], in_=tp[:])
                nc.sync.dma_start(
                    out=P_local[b * BLK : b * BLK + bs, :], in_=st[:bs, :]
                )

        def export_g(gT_tile, c0, w, li):
            """gT_tile [H, w] bf16 -> g_local rows [c0,c0+w) cols 0:H."""
            nblk_w = (w + BLK - 1) // BLK
            for k in range(nblk_w):
                b = (c0 + k * BLK) // BLK
                bs = bs_of(b)
                if bs <= 0:
                    break
                tp = ptr.tile([128, H], BF16, tag="ptr", name=f"gT{li}_{b}")
                nc.tensor.transpose(
                    out=tp[:, :], in_=gT_tile[:, k * BLK : k * BLK + 128],
                    identity=identb[:H, :H],
                )
                st = stage.tile([128, H], BF16, tag="gexp", name=f"gS{li}_{b}")
                nc.vector.tensor_copy(out=st[:], in_=tp[:])
                nc.sync.dma_start(
                    out=g_local[b * BLK : b * BLK + bs, :H], in_=st[:bs, :]
                )

        def stats_and_apply(layer, rep, w_l, residual, last):
            """AGG[:H] holds pre-norm agg (pre-innorm).  Compute
            h_lin = W^T(agg * innorm) (or just agg*innorm when w_l is None),
            BN stats, then apply scale/bias (+residual) + relu into HB[:H],
            and export P (or logits when last)."""
            li = f"{layer}_{rep}"
            stt = small.tile([H, 2 * NCH], F32, tag="stt", name=f"stt{li}")
            # --- pass 1: h_lin into AGG[:H], accumulate stats columns
            for ci in range(NCH):
                c0, w = chunk_cols(ci)
                irp = reppool.tile([128, CHUNK], BF16, tag="irp", name=f"ir{li}_{ci}")
                nc.sync.dma_start(out=irp[:H, :w], in_=ir_d[:H, c0 : c0 + w])
                ain = stage.tile([H, CHUNK], BF16, tag="ain", name=f"ain{li}_{ci}")
                nc.vector.tensor_tensor(
                    out=ain[:, :w], in0=AGG[:H, c0 : c0 + w], in1=irp[:H, :w], op=OP.mult
                )
                if w_l is not None:
                    hlp = pmm.tile([H, CHUNK], F32, tag="mm", name=f"hl{li}_{ci}")
                    nc.tensor.matmul(
                        out=hlp[:, :w], lhsT=w_l, rhs=ain[:, :w], start=True, stop=True
                    )
                    hl_ap = hlp[:, :w]
                else:
                    hl_ap = ain[:, :w]
                nc.vector.tensor_reduce(
                    out=stt[:, ci : ci + 1], in_=hl_ap, axis=mybir.AxisListType.X,
                    op=OP.add,
                )
                sq = stage.tile([H, CHUNK], BF16, tag="sq", name=f"sq{li}_{ci}")
                nc.scalar.activation(out=sq[:, :w], in_=hl_ap, func=AF.Square)
                nc.vector.tensor_reduce(
                    out=stt[:, NCH + ci : NCH + ci + 1], in_=sq[:, :w],
                    axis=mybir.AxisListType.X, op=OP.add,
                )
                nc.vector.tensor_copy(out=AGG[:H, c0 : c0 + w], in_=hl_ap)
            # --- stats AllReduce
            stl = small.tile([H, 2], F32, tag="stl", name=f"stl{li}")
            nc.vector.tensor_reduce(
                out=stl[:, 0:1], in_=stt[:, :NCH], axis=mybir.AxisListType.X, op=OP.add
            )
            nc.vector.tensor_reduce(
                out=stl[:, 1:2], in_=stt[:, NCH:], axis=mybir.AxisListType.X, op=OP.add
            )
            nc.sync.dma_start(out=st_local[:], in_=stl[:])
            nc.gpsimd.collective_compute(
                "AllReduce", OP.add, replica_groups=RG,
                ins=[st_local[:]], outs=[st_full[:]],
            )
            stg = small.tile([H, 2], F32, tag="stg", name=f"stg{li}")
            nc.sync.dma_start(out=stg[:], in_=st_full[:])
            mean = small.tile([H, 1], F32, tag="mean", name=f"mn{li}")
            nc.vector.tensor_scalar(
                out=mean[:], in0=stg[:, 0:1], scalar1=n_inv, scalar2=None, op0=OP.mult
            )
            e2 = small.tile([H, 1], F32, tag="e2", name=f"e2{li}")
            nc.vector.tensor_scalar(
                out=e2[:], in0=stg[:, 1:2], scalar1=n_inv, scalar2=None, op0=OP.mult
            )
            msq = small.tile([H, 1], F32, tag="msq", name=f"mq{li}")
            nc.vector.tensor_tensor(out=msq[:], in0=mean[:], in1=mean[:], op=OP.mult)
            var = small.tile([H, 1], F32, tag="var", name=f"vr{li}")
            nc.vector.tensor_tensor(out=var[:], in0=e2[:], in1=msq[:], op=OP.subtract)
            nc.vector.tensor_scalar(
                out=var[:], in0=var[:], scalar1=EPS, scalar2=None, op0=OP.add
            )
            rec = small.tile([H, 1], F32, tag="rec", name=f"rc{li}")
            nc.vector.reciprocal(out=rec[:], in_=var[:])
            rstd = small.tile([H, 1], F32, tag="rstd", name=f"rs{li}")
            nc.scalar.activation(out=rstd[:], in_=rec[:], func=AF.Sqrt)
            scale = small.tile([H, 1], F32, tag="scale", name=f"sc{li}")
            nc.vector.tensor_tensor(
                out=scale[:], in0=gam[:, layer : layer + 1], in1=rstd[:], op=OP.mult
            )
            tb = small.tile([H, 1], F32, tag="tb", name=f"tb{li}")
            nc.vector.tensor_tensor(out=tb[:], in0=mean[:], in1=scale[:], op=OP.mult)
            bias2 = small.tile([H, 1], F32, tag="bias2", name=f"b2{li}")
            nc.vector.tensor_tensor(
                out=bias2[:], in0=bet[:, layer : layer + 1], in1=tb[:], op=OP.subtract
            )
            # --- pass 2: apply + export
            for ci in range(NCH):
                c0, w = chunk_cols(ci)
                hn = stage.tile([H, CHUNK], BF16, tag="hn", name=f"hn{li}_{ci}")
                nc.scalar.activation(
                    out=hn[:, :w], in_=AGG[:H, c0 : c0 + w],
                    func=AF.Identity, scale=scale[:], bias=bias2[:],
                )
                if residual:
                    nc.vector.tensor_tensor(
                        out=hn[:, :w], in0=hn[:, :w], in1=HB[:H, c0 : c0 + w], op=OP.add
                    )
                nc.vector.tensor_scalar(
                    out=HB[:H, c0 : c0 + w], in0=hn[:, :w], scalar1=0.0, scalar2=None,
                    op0=OP.max,
                )
                if last:
                    lg = pmm.tile([C, CHUNK], F32, tag="mm", name=f"lg{li}_{ci}")
                    nc.tensor.matmul(
                        out=lg[:, :w], lhsT=wfc[:], rhs=HB[:H, c0 : c0 + w],
                        start=True, stop=True,
                    )
                    ot = stage.tile([C, CHUNK], F32, tag="ot", name=f"ot{li}_{ci}")
                    nc.scalar.activation(
                        out=ot[:, :w], in_=lg[:, :w], func=AF.Identity, bias=bfc[:]
                    )
                    wv = min(w, S - c0) if c0 < S else 0
                    if wv > 0:
                        nc.sync.dma_start(
                            out=outT_d[:, c0 : c0 + wv], in_=ot[:, :wv]
                        )
                else:
                    orp = reppool.tile([128, CHUNK], BF16, tag="orp", name=f"or{li}_{ci}")
                    nc.sync.dma_start(out=orp[:H, :w], in_=or_d[:H, c0 : c0 + w])
                    hs = stage.tile([H, CHUNK], BF16, tag="hs", name=f"hs{li}_{ci}")
                    nc.vector.tensor_tensor(
                        out=hs[:, :w], in0=HB[:H, c0 : c0 + w], in1=orp[:H, :w],
                        op=OP.mult,
                    )
                    export_P(HB[:H, c0 : c0 + w], hs, c0, w, f"{li}_{ci}")

        # ================== forward ==================
        for rep in range(reps):
            # ---- encoder + projection to z, build z table ----
            for ci in range(NCH):
                c0 = ci * CHUNK
                w = min(CHUNK, S - c0)
                if w <= 0:
                    break
                ft = stage.tile([IN, CHUNK], BF16, tag="ft", name=f"ft{rep}_{ci}")
                nc.sync.dma_start(out=ft[:, :w], in_=featT_d[:, c0 : c0 + w])
                e1p = pmm.tile([IN // 2, CHUNK], F32, tag="mm", name=f"e1p{rep}_{ci}")
                nc.tensor.matmul(out=e1p[:, :w], lhsT=wenc1[:], rhs=ft[:, :w], start=True, stop=True)
                e1 = stage.tile([IN // 2, CHUNK], BF16, tag="e1", name=f"e1{rep}_{ci}")
                nc.scalar.activation(out=e1[:, :w], in_=e1p[:, :w], func=AF.Relu, bias=benc1[:])
                e2p = pmm.tile([IN // 4, CHUNK], F32, tag="mm", name=f"e2p{rep}_{ci}")
                nc.tensor.matmul(out=e2p[:, :w], lhsT=wenc2[:], rhs=e1[:, :w], start=True, stop=True)
                e2s = stage.tile([IN // 4, CHUNK], BF16, tag="e2", name=f"e2{rep}_{ci}")
                nc.scalar.activation(out=e2s[:, :w], in_=e2p[:, :w], func=AF.Relu, bias=benc2[:])
                h0p = pmm.tile([IN, CHUNK], F32, tag="mm", name=f"h0p{rep}_{ci}")
                nc.tensor.matmul(out=h0p[:, :w], lhsT=wenc3[:], rhs=e2s[:, :w], start=True, stop=True)
                h0 = stage.tile([IN, CHUNK], BF16, tag="h0", name=f"h0{rep}_{ci}")
                nc.scalar.activation(out=h0[:, :w], in_=h0p[:, :w], func=AF.Identity, bias=benc3[:])
                zp = pmm.tile([H, CHUNK], F32, tag="mm", name=f"zp{rep}_{ci}")
                nc.tensor.matmul(out=zp[:, :w], lhsT=w0[:], rhs=h0[:, :w], start=True, stop=True)
                orp = reppool.tile([128, CHUNK], BF16, tag="orp", name=f"oz{rep}_{ci}")
                nc.sync.dma_start(out=orp[:H, :w], in_=or_d[:H, c0 : c0 + w])
                gT = stage.tile([H, CHUNK], BF16, tag="gT", name=f"gT{rep}_{ci}")
                nc.vector.tensor_tensor(
                    out=gT[:, :w], in0=zp[:, :w], in1=orp[:H, :w], op=OP.mult
                )
                export_g(gT, c0, w, f"z{rep}_{ci}")

            if rep == 0 and DUMPG:
                nc.sync.dma_start(out=dbgz_d[:], in_=g_local[:])
            if STAGE < 2:
                continue
            nc.gpsimd.collective_compute(
                "AllGather", OP.bypass, replica_groups=RG,
                ins=[g_local[:]], outs=[g_full[:]],
            )
            if STAGE < 3:
                continue
            spmm(g_full, False, "copy", rep, "l0")
            if rep == 0:
                dump(AGG[:])
            if STAGE < 4:
                continue
            stats_and_apply(0, rep, None, residual=False, last=False)
            if rep == 0:
                dump(HB[:])

            if STAGE < 5:
                continue
            # ---- layers 1..L-1 ----
            for layer in range(1, L):
                last = layer == L - 1
                w_l = wrest[:, (layer - 1) * H : layer * H]
                nc.gpsimd.collective_compute(
                    "AllGather", OP.bypass, replica_groups=RG,
                    ins=[P_local[:]], outs=[P_full[:]],
                )
                spmm(P_full, True, "passA", rep, f"a{layer}")
                if rep == 0 and layer == 1:
                    dump(HB[:])
                    dump(AGG[:])

                # attention + delta table
                for ci in range(NCH):
                    c0, w = chunk_cols(ci)
                    li = f"at{layer}_{rep}_{ci}"
                    rr = reppool.tile([128, CHUNK], BF16, tag="rr", name=f"rr{li}")
                    nc.sync.dma_start(out=rr[:, :w], in_=rr_d[:, c0 : c0 + w])
                    # nb = S_h * rindeg  (in place, rows H:128)
                    nc.vector.tensor_tensor(
                        out=HB[H:128, c0 : c0 + w], in0=HB[H:128, c0 : c0 + w],
                        in1=rr[H:128, :w], op=OP.mult,
                    )
                    a1p = pmm.tile([H, CHUNK], F32, tag="mm", name=f"a1p{li}")
                    nc.tensor.matmul(
                        out=a1p[:, :w], lhsT=watt1[:], rhs=HB[:, c0 : c0 + w],
                        start=True, stop=True,
                    )
                    a1 = stage.tile([H, CHUNK], BF16, tag="a1", name=f"a1{li}")
                    nc.scalar.activation(out=a1[:, :w], in_=a1p[:, :w], func=AF.Relu, bias=batt1[:])
                    a2pt = pmm.tile([128, CHUNK], F32, tag="mm", name=f"a2p{li}")
                    a2p = a2pt[0:1, :]
                    nc.tensor.matmul(
                        out=a2p[:, :w], lhsT=watt2[:], rhs=a1[:, :w], start=True, stop=True
                    )
                    a2 = stage.tile([1, CHUNK], BF16, tag="a2", name=f"a2{li}")
                    nc.scalar.activation(out=a2[:, :w], in_=a2p[:, :w], func=AF.Sigmoid, bias=batt2[:])
                    arp = pmm.tile([128, CHUNK], F32, tag="mm", name=f"arp{li}")
                    nc.tensor.matmul(
                        out=arp[:, :w], lhsT=ones1[:], rhs=a2[:, :w], start=True, stop=True
                    )
                    # arp rows are all identical, so read it at base 0 (PSUM)
                    # against the SB operand at base 64 — mixed-space bases.
                    anb = stage.tile([H, CHUNK], BF16, tag="anb", name=f"anb{li}")
                    nc.vector.tensor_tensor(
                        out=anb[:, :w], in0=arp[0:H, :w], in1=HB[H:128, c0 : c0 + w],
                        op=OP.mult,
                    )
                    nc.vector.tensor_tensor(
                        out=HB[:H, c0 : c0 + w], in0=HB[:H, c0 : c0 + w],
                        in1=anb[:, :w], op=OP.add,
                    )
                    orp = reppool.tile([128, CHUNK], BF16, tag="orp", name=f"oa{li}")
                    nc.sync.dma_start(out=orp[:H, :w], in_=or_d[:H, c0 : c0 + w])
                    gT = stage.tile([H, CHUNK], BF16, tag="gT", name=f"gTa{li}")
                    nc.vector.tensor_tensor(
                        out=gT[:, :w], in0=anb[:, :w], in1=orp[:H, :w], op=OP.mult
                    )
                    export_g(gT, c0, w, li)

                if rep == 0 and layer == 1:
                    dump(HB[:])
                nc.gpsimd.collective_compute(
                    "AllGather", OP.bypass, replica_groups=RG,
                    ins=[g_local[:]], outs=[g_full[:]],
                )
                spmm(g_full, False, "add", rep, f"b{layer}")
                if rep == 0 and layer == 1:
                    dump(AGG[:])
                stats_and_apply(layer, rep, w_l, residual=True, last=last)
                if rep == 0 and layer == 1:
                    dump(HB[:])

        ctx.close()

    return nc


# --------------------------------------------------------------------------
def _make_in_maps(meta, inputs, IN, H, C, L):
    N, S, S_pad = meta["N"], meta["S"], meta["S_pad"]
    f = lambda x: np.ascontiguousarray(np.asarray(x, dtype=np.float32))
    fb = lambda x: np.ascontiguousarray(np.asarray(x, dtype=np.float32).astype(bf16))
    feats = np.asarray(inputs["features"], np.float32)
    W_rest = f(inputs["W_rest"])
    iota = np.tile(np.arange(128, dtype=np.float32)[None, :], (128, 1))
    shared = dict(
        iota=iota.astype(bf16),
        identb=np.eye(128, dtype=np.float32).astype(bf16),
        ones1=np.ones((1, 128), np.float32).astype(bf16),
        wenc1=fb(inputs["enc_W1"]),
        benc1=f(inputs["enc_b1"])[:, None],
        wenc2=fb(inputs["enc_W2"]),
        benc2=f(inputs["enc_b2"])[:, None],
        wenc3=fb(inputs["enc_W3"]),
        benc3=f(inputs["enc_b3"])[:, None],
        w0=fb(inputs["W0"]),
        wrest=np.ascontiguousarray(
            W_rest.transpose(1, 0, 2).reshape(W_rest.shape[1], -1)
        ).astype(bf16),
        gam=np.ascontiguousarray(f(inputs["gamma"]).T),
        bet=np.ascontiguousarray(f(inputs["beta"]).T),
        watt1=fb(inputs["att_W1"]),
        batt1=f(inputs["att_b1"])[:, None],
        watt2=fb(inputs["att_W2"]),
        batt2=f(inputs["att_b2"])[:, None],
        wfc=fb(inputs["fc_W"]),
        bfc=f(inputs["fc_b"])[:, None],
    )
    in_maps = []
    for c in range(NCORES):
        m = dict(shared)
        m["featT"] = np.ascontiguousarray(feats[c * S : (c + 1) * S].T).astype(bf16)
        m["idx"] = np.ascontiguousarray(meta["idx"][c])
        m["seg"] = np.ascontiguousarray(meta["seg"][c]).astype(bf16)
        m["rrep"] = np.ascontiguousarray(meta["rindeg_rep"][c])
        m["irep"] = np.ascontiguousarray(meta["innorm_rep"][c])
        m["orep"] = np.ascontiguousarray(meta["outnorm_rep"][c])
        in_maps.append(m)
    return in_maps


def _prep_all(inputs, reps=1):
    feats = np.asarray(inputs["features"])
    N, IN = feats.shape
    H = np.asarray(inputs["W0"]).shape[1]
    C = np.asarray(inputs["fc_W"]).shape[1]
    L = np.asarray(inputs["gamma"]).shape[0]
    src = np.asarray(inputs["src"]).astype(np.int64)
    dst = np.asarray(inputs["dst"]).astype(np.int64)
    meta = _preprocess(N, src, dst)
    nc = _build_program(meta, IN, H, C, L, reps=reps)
    nc.compile()
    _split_excess_waits(nc)
    in_maps = _make_in_maps(meta, inputs, IN, H, C, L)
    return meta, nc, in_maps, (IN, H, C, L)


def kernel(**inputs):
    meta, nc, in_maps, (IN, H, C, L) = _prep_all(inputs, reps=1)
    res = run_bass_kernel_spmd(nc, in_maps, list(range(NCORES)))
    S, N = meta["S"], meta["N"]
    out = np.empty((N, C), np.float32)
    for c in range(NCORES):
        out[c * S : (c + 1) * S] = res.results[c]["outT"].T
    return out
